# revision 27
# baseline (speedup 1.0000x reference)
"""Trainium2 Bass kernel v3: single-head causal attention, bf16 datapath.

Problem: x[4,4096,128]; Q/K/V linear projections (W [in,out] layout, +bias);
scores = QK^T/sqrt(128) with causal mask; softmax; out = P @ V.

Sharding (8 cores = 4 batches x 2), identical to v2:
  core (b, h):
    triangle: queries [2048h, 2048h+2048) of batch b attending causally
        within the same range.
    rectangle: queries [2048, 4096) attending to kv rows [1024h, 1024h+1024).
  Union over a batch's two cores covers the causal set exactly once.

Softmax without max subtraction (scores ~N(0,1)); cross-core merge is
linear: host sums unnormalized outputs oT and denominators, then divides.

v3 changes vs v2 (cost-model-driven):
  - bf16 everywhere on the datapath (x, W, Q^T, K^T, V, P~, mask) instead of
    fp32r: halves DMA traffic and SBUF; matmul rate identical (1 cyc/row).
  - NO on-device softmax denominator: the PE "ones" matmuls (l) are gone.
    Instead the DVE accumulates each chunk's exp'd probability tiles into
    ptsum[128, 512] (bf16) and the host reduces the 128 kv-lanes. This takes
    ~14us of matmul time off the PE (the former bottleneck) for ~20us of
    otherwise-idle DVE time.
  - projections interleaved with attention chunks: the ACT engine (exp) and
    the DMA start ~2.5us into the kernel instead of after the whole
    projection phase.
  - epilogue: po (PSUM) -> bf16 SBUF copy -> DMA; ptsum DMAs straight from
    SBUF. Outputs oT[D,T] bf16 + lsum[D,T] bf16.

Per-unit steady state (pair of kv tiles x 512 queries):
  PE: ST pair (1024 rows) + AV pair (1024 rows) ~ 854ns @full clock
  ACT: exp [128, <=1024] ~ 1038ns  <- pacer
  DVE: 2 presum adds ~ 654ns (+ proj copies early on)
Engine budgets/core: PE ~37us (incl ~4us p-state ramp), ACT ~35us, DVE ~38us.
"""

import math
import sys

import numpy as np

sys.path.insert(0, "/opt/trn_rl_repo")

import ml_dtypes  # noqa: E402

import concourse.bass as bass  # noqa: E402
import concourse.mybir as mybir  # noqa: E402
from concourse.tile import TileContext  # noqa: E402

B, T, D = 4, 4096, 128
HALF = T // 2          # 2048 queries per triangle
NCHUNK = 8             # 8 chunks of 512 query slots per core (4 tri + 4 rect)
CHUNK = 512
KV_TRI_TILES = 16      # triangle kv tiles (2048 rows)
KV_RECT_TILES = 8      # rectangle kv tiles (1024 rows)
KV_TILES = KV_TRI_TILES + KV_RECT_TILES          # 24 tiles = 3072 kv rows
NEG = -1.0e5           # additive mask value; exp(NEG) == 0.0

F32 = mybir.dt.float32
BF16 = mybir.dt.bfloat16
bfloat16 = ml_dtypes.bfloat16


def build_nc(legalize=True):
    nc = bass.Bass()

    xtq_d = nc.declare_dram_parameter("xTq", [D, T], BF16, isOutput=False)
    xtk_d = nc.declare_dram_parameter("xTk", [D, KV_TILES * 128], BF16, isOutput=False)
    wa_d = nc.declare_dram_parameter("wpackA", [D, 258], BF16, isOutput=False)
    wb_d = nc.declare_dram_parameter("wpackB", [D, 2 * D + 4 * 128], BF16,
                                     isOutput=False)

    ot_d = nc.declare_dram_parameter("oT", [D, T], BF16, isOutput=True)
    ls_d = nc.declare_dram_parameter("lsum", [D, T], BF16, isOutput=True)

    # attention chunk -> kv tile order: diagonal tiles ascending (valid-col
    # prefix logic needs the widest first), then full tiles in reverse so the
    # masked diagonal work lands right after the chunk's projections.
    def _tri_ts(c):
        fulls = list(range(0, 4 * c))[::-1]
        diag = list(range(4 * c, 4 * c + 4))
        # keep one full pair after the diagonals so the chunk's last exp is
        # full-sized and covers the boundary refill; chunk 3 runs last in the
        # kernel and keeps diag-last so the kernel tail stays small
        if len(fulls) >= 2 and c != 3:
            return fulls[:-2] + diag + fulls[-2:]
        return fulls + diag

    chunk_ts = [_tri_ts(c) for c in range(4)] + \
               [list(range(16, 24))[::-1] for _ in range(4)]

    with TileContext(nc) as tc:
        with (
            tc.tile_pool(name="sb", bufs=1) as sb,          # resident tensors
            tc.tile_pool(name="stp", bufs=2, space="PSUM") as stp,   # 4 banks
            tc.tile_pool(name="prj", bufs=2, space="PSUM") as prj,   # 2 banks
            tc.tile_pool(name="op", bufs=2, space="PSUM") as op,     # 2 banks
            tc.tile_pool(name="ptp", bufs=1) as ptp,
            tc.tile_pool(name="pts", bufs=3) as pts,
            tc.tile_pool(name="tmpp", bufs=6) as tmpp,
            tc.tile_pool(name="osb", bufs=3) as osb,
        ):
            # ---- resident SBUF tensors; DMAs issued in first-use order ----
            wa = sb.tile([D, 258], BF16)
            nc.sync.dma_start(out=wa, in_=wa_d[:, :])
            xtk = sb.tile([D, KV_TILES * 128], BF16)
            nc.sync.dma_start(out=xtk[:, 0:CHUNK], in_=xtk_d[:, 0:CHUNK])
            xtq = sb.tile([D, T], BF16)
            nc.sync.dma_start(out=xtq[:, 0:CHUNK], in_=xtq_d[:, 0:CHUNK])
            wb = sb.tile([D, 2 * D + 4 * 128], BF16)
            nc.sync.dma_start(out=wb, in_=wb_d[:, :])
            # bulk x, split and ordered by first use
            nc.sync.dma_start(out=xtk[:, CHUNK:2 * CHUNK],
                              in_=xtk_d[:, CHUNK:2 * CHUNK])
            nc.sync.dma_start(out=xtk[:, 2 * CHUNK:4 * CHUNK],
                              in_=xtk_d[:, 2 * CHUNK:4 * CHUNK])
            nc.sync.dma_start(out=xtq[:, CHUNK:4 * CHUNK],
                              in_=xtq_d[:, CHUNK:4 * CHUNK])
            nc.sync.dma_start(out=xtk[:, 4 * CHUNK:], in_=xtk_d[:, 4 * CHUNK:])
            nc.sync.dma_start(out=xtq[:, 4 * CHUNK:], in_=xtq_d[:, 4 * CHUNK:])
            wk = wa[:, 0:D]
            wq = wa[:, D:2 * D]
            bq = wa[:, 2 * D:2 * D + 2].bitcast(F32)
            wv = wb[:, 0:D]
            ident = wb[:, D:2 * D]
            msk = wb[:, 2 * D:]

            qt = sb.tile([D, T], BF16)                # Q^T (scaled, biased)
            kt = sb.tile([D, KV_TILES * 128], BF16)   # K^T
            vsb = sb.tile([D, KV_TILES * 128], BF16)  # V tiles [kvrow, e]

            pt_tiles = [ptp.tile([D, 2 * CHUNK], BF16, name=f"pt{i}")
                        for i in range(4)]
            npt = [0]
            ntmp = [0]

            # ---- emission helpers ----
            def eng_copy(eng, out, in_):
                if eng is nc.scalar:
                    nc.scalar.copy(out, in_)
                else:
                    (eng or nc.vector).tensor_copy(out, in_)

            def proj_K(g, eng=None, half=None):
                for h in ((0, 1) if half is None else (half,)):
                    sl = slice(g * CHUNK + h * 256, g * CHUNK + (h + 1) * 256)
                    ps = prj.tile([D, 256], F32, tag="prj", name="prjk")
                    nc.tensor.matmul(ps, wk, xtk[:, sl],
                                     start=True, stop=True,
                                     skip_group_check=True)
                    eng_copy(eng, kt[:, sl], ps)

            def proj_V(g, eng=None, half=None):
                for h in ((0, 1) if half is None else (half,)):
                    ps = prj.tile([D, 256], F32, tag="prj", name="prjv")
                    for jj in range(2):
                        t = 4 * g + 2 * h + jj
                        nc.tensor.matmul(
                            ps[:, jj * 128:(jj + 1) * 128],
                            xtk[:, t * 128:(t + 1) * 128], wv,
                            start=True, stop=True, skip_group_check=True)
                    sl = slice(g * CHUNK + h * 256, g * CHUNK + (h + 1) * 256)
                    eng_copy(eng, vsb[:, sl], ps)

            def proj_Q(g, eng=None, half=None):
                for h in ((0, 1) if half is None else (half,)):
                    sl = slice(g * CHUNK + h * 256, g * CHUNK + (h + 1) * 256)
                    ps = prj.tile([D, 256], F32, tag="prj", name="prjq")
                    nc.tensor.matmul(ps, wq, xtq[:, sl],
                                     start=True, stop=True,
                                     skip_group_check=True)
                    if eng is nc.scalar:
                        nc.scalar.activation(
                            qt[:, sl], ps,
                            mybir.ActivationFunctionType.Identity, bias=bq)
                    else:
                        nc.vector.tensor_scalar_add(qt[:, sl], ps, bq)

            state = {"pend": None, "acc": {}, "psum": {}, "epi": [],
                     "projplan": {}, "uidx": 0}

            def emit_epilogue(final=False):
                c, po, psum_t = state["epi"].pop(0)
                qsl = slice(c * CHUNK, (c + 1) * CHUNK)
                # lsum is ready at presum-chain end; ship it before po's copy
                nc.sync.dma_start(out=ls_d[:, qsl], in_=psum_t)
                ob = osb.tile([D, CHUNK], BF16, tag="ob", name="ob")
                if final:
                    nc.scalar.copy(ob, po)   # ACT is idle after the last exp
                else:
                    nc.vector.tensor_copy(ob, po)
                nc.sync.dma_start(out=ot_d[:, qsl], in_=ob)

            def emit_av(pend):
                c, ts, pair, is_last, pt, los, cum = pend
                if c not in state["acc"]:
                    state["acc"][c] = op.tile([D, CHUNK], F32, tag="po",
                                              name="po")
                po = state["acc"][c]
                for i, t in enumerate(pair):
                    lo = los[i]
                    ptc = pt[:, cum[i]:cum[i + 1]]
                    nc.tensor.matmul(
                        po[:, lo:], vsb[:, t * 128:(t + 1) * 128], ptc,
                        start=(t == ts[0]), stop=(t == ts[-1]),
                        skip_group_check=True)
                if is_last:
                    state["epi"].append((c, po, state["psum"][c]))
                    del state["acc"][c]
                    del state["psum"][c]

            def emit_unit(c, ts, pair, is_last, ui):
                """ST + mask (PE), exp (ACT), presum (DVE), delayed AV."""
                if state["epi"]:
                    emit_epilogue()
                los = [128 * (t - 4 * c) if c < 4 and t >= 4 * c else 0
                       for t in pair]
                # pack each tile's valid columns contiguously: tile i starts
                # at cum[i], width 512-los[i]; one exp covers [0:cum_end]
                cum = [0]
                for lo in los:
                    cum.append(cum[-1] + CHUNK - lo)
                st = stp.tile([D, 2 * CHUNK], F32, tag="st", name="st")
                for i, t in enumerate(pair):
                    lo = los[i]
                    nc.tensor.matmul(
                        st[:, cum[i]:cum[i + 1]],
                        kt[:, t * 128:(t + 1) * 128],
                        qt[:, c * CHUNK + lo:(c + 1) * CHUNK],
                        start=True, stop=True, skip_group_check=True)
                    if c < 4 and t >= 4 * c:
                        nc.tensor.matmul(
                            st[:, cum[i]:cum[i] + 128],
                            ident,
                            msk[:, (t - 4 * c) * 128:(t - 4 * c + 1) * 128],
                            start=False, stop=True, skip_group_check=True)
                pt = pt_tiles[npt[0] % 4]
                npt[0] += 1
                nc.scalar.activation(
                    pt[:, 0:cum[-1]], st[:, 0:cum[-1]],
                    mybir.ActivationFunctionType.Exp)

                # ---- denominator presum into ptsum (bf16) ----
                # tree: tmp = ptA + ptB (Pool/DVE alternating), then the
                # short serial chain ptsum += tmp stays on DVE.
                if ui == 0:
                    psum_t = pts.tile([D, CHUNK], BF16, tag="pts",
                                      name="pts")
                    state["psum"][c] = psum_t
                psum_t = state["psum"][c]
                add = mybir.AluOpType.add
                if los[1] > 0:
                    # diagonal unit: small windowed ops straight on DVE
                    if ui == 0:
                        nc.vector.tensor_copy(
                            psum_t[:, 0:los[1]], pt[:, 0:los[1]])
                        nc.vector.tensor_tensor(
                            out=psum_t[:, los[1]:],
                            in0=pt[:, los[1]:cum[1]],
                            in1=pt[:, cum[1]:cum[2]], op=add)
                    else:
                        for i in range(len(pair)):
                            lo = los[i]
                            nc.vector.tensor_tensor(
                                out=psum_t[:, lo:], in0=psum_t[:, lo:],
                                in1=pt[:, cum[i]:cum[i + 1]],
                                op=add)
                elif ui == 0:
                    nc.vector.tensor_tensor(
                        out=psum_t, in0=pt[:, 0:CHUNK],
                        in1=pt[:, CHUNK:], op=add)
                else:
                    tmp = tmpp.tile([D, CHUNK], BF16, tag="tmp", name="tmp")
                    eng = nc.gpsimd if ntmp[0] % 2 == 0 else nc.vector
                    ntmp[0] += 1
                    eng.tensor_tensor(out=tmp, in0=pt[:, 0:CHUNK],
                                      in1=pt[:, CHUNK:], op=add)
                    nc.vector.tensor_tensor(out=psum_t, in0=psum_t,
                                            in1=tmp, op=add)

                for fn in state["projplan"].get(state["uidx"], ()):
                    fn()
                state["uidx"] += 1
                prev, state["pend"] = state["pend"], (c, ts, pair, is_last,
                                                      pt, los, cum)
                if prev is not None:
                    emit_av(prev)

            def emit_chunk(c):
                ts = chunk_ts[c]
                pairs = [ts[i:i + 2] for i in range(0, len(ts), 2)]
                for pi, pair in enumerate(pairs):
                    emit_unit(c, ts, pair, pi == len(pairs) - 1, pi)

            # ---- PE warmup: back-to-back dummy matmuls from t~0.6us keep
            # the tensor engine's p-state ramp running during the input DMA
            # wait, so real projections start at full clock. Operands are
            # uninitialized SBUF (qt is first WRITTEN later); results land in
            # prj tiles that are overwritten with start=True. ----
            scr = sb.tile([D, 384], BF16)
            nc.gpsimd.memset(scr, 0.0)
            for _ in range(15):
                wup = prj.tile([D, CHUNK], F32, tag="prj", name="wup")
                nc.tensor.matmul(wup[:, 0:256], scr[:, 0:D],
                                 scr[:, D:D + 256],
                                 start=True, stop=True, skip_group_check=True)

            # ---- interleaved schedule ----
            # Group-0 projections run up front (qt0's copy rides the idle
            # ACT so kt0 on DVE lands in parallel). Every other projection
            # piece is queued and dropped one-per-unit into the attention
            # stream, ordered by first use. Chunk 3 runs last so the kernel
            # tail is a small masked diagonal unit, not a full one.
            proj_K(0, "split"); proj_Q(0, "split")
            # static plan: unit index -> proj half-pieces to emit there,
            # each ~2-4 units ahead of first use (chunk order 0,1,2,4,5,6,7,3)
            fns = {"Q": proj_Q, "K": proj_K, "V": proj_V}
            plan = {
                0: [("Q", 1, 0), ("Q", 1, 1)],
                1: [("K", 1, 0)],
                2: [("K", 1, 1), ("V", 1, 0)],
                3: [("V", 1, 1), ("Q", 2, 0)],
                4: [("Q", 2, 1)],
                6: [("K", 2, 0)],
                7: [("K", 2, 1), ("V", 2, 0)],
                8: [("V", 2, 1), ("Q", 4, 0)],
                9: [("Q", 4, 1), ("K", 5, 1)],
                10: [("K", 5, 0), ("V", 5, 1)],
                11: [("V", 5, 0), ("K", 4, 1)],
                12: [("K", 4, 0), ("V", 4, 1)],
                13: [("V", 4, 0), ("Q", 5, 0)],
                14: [("Q", 5, 1)],
                16: [("Q", 6, 0)],
                17: [("Q", 6, 1)],
                20: [("Q", 7, 0)],
                21: [("Q", 7, 1)],
                26: [("Q", 3, 0)],
                27: [("Q", 3, 1)],
                28: [("K", 3, 0)],
                29: [("K", 3, 1)],
                30: [("V", 3, 0)],
                31: [("V", 3, 1)],
            }
            state["projplan"] = {
                u: [(lambda f=fns[k], g=g, h=h: f(g, None, h))
                    for k, g, h in pieces]
                for u, pieces in plan.items()
            }
            for c in (0, 1, 2, 4, 5, 6, 7, 3):
                emit_chunk(c)
            emit_av(state["pend"])
            while state["epi"]:
                emit_epilogue(final=len(state["epi"]) == 1)

    if legalize:
        _legalize_multiwaits(nc)
    nc.finalize()
    return nc


def _legalize_multiwaits(nc):
    """Hardware instruction structs accept at most ONE sync wait. Move all
    but the last wait onto single-wait same-engine NoOps inserted before the
    instruction (same-engine program order preserves semantics)."""
    for fn in nc.m.functions:
        for blk in fn.blocks:
            insts = blk.instructions
            out = []
            for inst in insts:
                si = inst.sync_info
                if si is not None and si.on_wait and len(si.on_wait) >= 2:
                    waits = list(si.on_wait)
                    for w in waits[:-1]:
                        out.append(mybir.InstNoOp(
                            name=nc.get_next_instruction_name(),
                            engine=inst.engine,
                            bass_nofuse=True,
                            sync_info=mybir.SyncInfo(
                                on_wait=[w], on_update=[]),
                        ))
                    inst.sync_info = mybir.SyncInfo(
                        on_wait=[waits[-1]],
                        on_update=list(si.on_update or []))
                out.append(inst)
            insts[:] = out


_NC_CACHE = {}


def get_nc(legalize=True):
    key = ("nc", legalize)
    if key not in _NC_CACHE:
        _NC_CACHE[key] = build_nc(legalize)
    return _NC_CACHE[key]


def make_core_inputs(x, Wq, bq, Wk, bk, Wv, bv):
    """Per-core input maps (host-side sharding). bk drops out of softmax;
    bv is applied on the host."""
    s = 1.0 / math.sqrt(D)
    wq_s = (np.asarray(Wq, np.float32) * s).astype(bfloat16)
    bq_s = (np.asarray(bq, np.float32) * s)
    wk = np.asarray(Wk, np.float32).astype(bfloat16)
    wv = np.asarray(Wv, np.float32).astype(bfloat16)

    qp = np.arange(128)[None, :]
    kk = np.arange(128)[:, None]
    # per-m staircase band (query cols [128m, 128m+128) relative part)
    msk = np.concatenate(
        [np.where(qp >= kk, 0.0, NEG) for m in range(4)],
        axis=1).astype(bfloat16)
    ident = np.eye(D, dtype=np.float32).astype(bfloat16)
    bq_bits = np.ascontiguousarray(bq_s[:, None]).view(bfloat16)
    wpackA = np.concatenate([wk, wq_s, bq_bits], axis=1)
    wpackB = np.concatenate([wv, ident, msk], axis=1)

    x = np.asarray(x, dtype=np.float32)
    in_maps = []
    for core in range(8):
        b, h = core // 2, core % 2
        xb = x[b]                                   # [4096, 128]
        tri = xb[h * HALF:(h + 1) * HALF]           # [2048, 128]
        rect_q = xb[HALF:]                          # [2048, 128]
        rect_kv = xb[h * 1024:(h + 1) * 1024]       # [1024, 128]
        xtq = np.ascontiguousarray(
            np.concatenate([tri, rect_q], axis=0).T).astype(bfloat16)
        xtk = np.ascontiguousarray(
            np.concatenate([tri, rect_kv], axis=0).T).astype(bfloat16)
        in_maps.append({
            "xTq": xtq, "xTk": xtk, "wpackA": wpackA, "wpackB": wpackB,
        })
    return in_maps


def merge_outputs(results, bv):
    """Gather per-core (oT, lsum) into the full [B, T, D] output."""
    bv = np.asarray(bv, dtype=np.float32)
    out = np.empty((B, T, D), np.float32)
    for b in range(B):
        lo, hi = results[2 * b], results[2 * b + 1]
        lo_oT = np.asarray(lo["oT"], np.float64)
        hi_oT = np.asarray(hi["oT"], np.float64)
        lo_l = np.asarray(lo["lsum"], np.float64).sum(axis=0)   # [T]
        hi_l = np.asarray(hi["lsum"], np.float64).sum(axis=0)   # [T]
        O = np.zeros((T, D), np.float64)
        L = np.zeros(T, np.float64)
        O[:HALF] += lo_oT[:, :HALF].T
        L[:HALF] += lo_l[:HALF]
        O[HALF:] += hi_oT[:, :HALF].T
        L[HALF:] += hi_l[:HALF]
        O[HALF:] += lo_oT[:, HALF:].T
        L[HALF:] += lo_l[HALF:]
        O[HALF:] += hi_oT[:, HALF:].T
        L[HALF:] += hi_l[HALF:]
        out[b] = (O / L[:, None]).astype(np.float32) + bv
    return out


def run_per_core(nc, in_maps, threads=True):
    """Run the same single-core program on each NeuronCore with its own
    inputs (independent dispatch; the cores share no collectives)."""
    import jax
    from concourse import bass2jax

    devices = jax.devices()[:len(in_maps)]

    def one(i):
        with jax.default_device(devices[i]):
            return bass2jax.run_bass_via_pjrt(nc, [in_maps[i]], n_cores=1)[0]

    if threads:
        from concurrent.futures import ThreadPoolExecutor
        first = one(0)
        with ThreadPoolExecutor(max_workers=7) as ex:
            rest = list(ex.map(one, range(1, len(in_maps))))
        return [first] + rest
    return [one(i) for i in range(len(in_maps))]


def kernel(x, Wq, bq, Wk, bk, Wv, bv, _trace=False):
    from concourse.bass_utils import axon_active, run_bass_kernel_spmd

    nc = get_nc()
    in_maps = make_core_inputs(x, Wq, bq, Wk, bk, Wv, bv)
    if axon_active():
        results = run_per_core(nc, in_maps)
    else:
        res = run_bass_kernel_spmd(nc, in_maps, list(range(8)), trace=_trace)
        kernel.last_result = res
        results = res.results
    out = merge_outputs(results, bv)
    return out


# revision 28
# speedup vs baseline: 1.0031x; 1.0031x over previous
"""Trainium2 Bass kernel v3: single-head causal attention, bf16 datapath.

Problem: x[4,4096,128]; Q/K/V linear projections (W [in,out] layout, +bias);
scores = QK^T/sqrt(128) with causal mask; softmax; out = P @ V.

Sharding (8 cores = 4 batches x 2), identical to v2:
  core (b, h):
    triangle: queries [2048h, 2048h+2048) of batch b attending causally
        within the same range.
    rectangle: queries [2048, 4096) attending to kv rows [1024h, 1024h+1024).
  Union over a batch's two cores covers the causal set exactly once.

Softmax without max subtraction (scores ~N(0,1)); cross-core merge is
linear: host sums unnormalized outputs oT and denominators, then divides.

v3 changes vs v2 (cost-model-driven):
  - bf16 everywhere on the datapath (x, W, Q^T, K^T, V, P~, mask) instead of
    fp32r: halves DMA traffic and SBUF; matmul rate identical (1 cyc/row).
  - NO on-device softmax denominator: the PE "ones" matmuls (l) are gone.
    Instead the DVE accumulates each chunk's exp'd probability tiles into
    ptsum[128, 512] (bf16) and the host reduces the 128 kv-lanes. This takes
    ~14us of matmul time off the PE (the former bottleneck) for ~20us of
    otherwise-idle DVE time.
  - projections interleaved with attention chunks: the ACT engine (exp) and
    the DMA start ~2.5us into the kernel instead of after the whole
    projection phase.
  - epilogue: po (PSUM) -> bf16 SBUF copy -> DMA; ptsum DMAs straight from
    SBUF. Outputs oT[D,T] bf16 + lsum[D,T] bf16.

Per-unit steady state (pair of kv tiles x 512 queries):
  PE: ST pair (1024 rows) + AV pair (1024 rows) ~ 854ns @full clock
  ACT: exp [128, <=1024] ~ 1038ns  <- pacer
  DVE: 2 presum adds ~ 654ns (+ proj copies early on)
Engine budgets/core: PE ~37us (incl ~4us p-state ramp), ACT ~35us, DVE ~38us.
"""

import math
import sys

import numpy as np

sys.path.insert(0, "/opt/trn_rl_repo")

import ml_dtypes  # noqa: E402

import concourse.bass as bass  # noqa: E402
import concourse.mybir as mybir  # noqa: E402
from concourse.tile import TileContext  # noqa: E402

B, T, D = 4, 4096, 128
HALF = T // 2          # 2048 queries per triangle
NCHUNK = 8             # 8 chunks of 512 query slots per core (4 tri + 4 rect)
CHUNK = 512
KV_TRI_TILES = 16      # triangle kv tiles (2048 rows)
KV_RECT_TILES = 8      # rectangle kv tiles (1024 rows)
KV_TILES = KV_TRI_TILES + KV_RECT_TILES          # 24 tiles = 3072 kv rows
NEG = -1.0e5           # additive mask value; exp(NEG) == 0.0

F32 = mybir.dt.float32
BF16 = mybir.dt.bfloat16
bfloat16 = ml_dtypes.bfloat16


def build_nc(legalize=True):
    nc = bass.Bass()

    xtq_d = nc.declare_dram_parameter("xTq", [D, T], BF16, isOutput=False)
    xtk_d = nc.declare_dram_parameter("xTk", [D, KV_TILES * 128], BF16, isOutput=False)
    wa_d = nc.declare_dram_parameter("wpackA", [D, 258], BF16, isOutput=False)
    wb_d = nc.declare_dram_parameter("wpackB", [D, 2 * D + 4 * 128], BF16,
                                     isOutput=False)

    ot_d = nc.declare_dram_parameter("oT", [D, T], BF16, isOutput=True)
    ls_d = nc.declare_dram_parameter("lsum", [D, T], BF16, isOutput=True)

    # attention chunk -> kv tile order: diagonal tiles ascending (valid-col
    # prefix logic needs the widest first), then full tiles in reverse so the
    # masked diagonal work lands right after the chunk's projections.
    def _tri_ts(c):
        fulls = list(range(0, 4 * c))[::-1]
        diag = list(range(4 * c, 4 * c + 4))
        # keep one full pair after the diagonals so the chunk's last exp is
        # full-sized and covers the boundary refill; chunk 3 runs last in the
        # kernel and keeps diag-last so the kernel tail stays small
        if len(fulls) >= 2 and c != 3:
            return fulls[:-2] + diag + fulls[-2:]
        return fulls + diag

    chunk_ts = [_tri_ts(c) for c in range(4)] + \
               [list(range(16, 24))[::-1] for _ in range(4)]

    with TileContext(nc) as tc:
        with (
            tc.tile_pool(name="sb", bufs=1) as sb,          # resident tensors
            tc.tile_pool(name="stp", bufs=2, space="PSUM") as stp,   # 4 banks
            tc.tile_pool(name="prj", bufs=2, space="PSUM") as prj,   # 2 banks
            tc.tile_pool(name="op", bufs=2, space="PSUM") as op,     # 2 banks
            tc.tile_pool(name="ptp", bufs=1) as ptp,
            tc.tile_pool(name="pts", bufs=3) as pts,
            tc.tile_pool(name="tmpp", bufs=6) as tmpp,
            tc.tile_pool(name="osb", bufs=3) as osb,
        ):
            # ---- resident SBUF tensors; DMAs issued in first-use order ----
            wa = sb.tile([D, 258], BF16)
            nc.sync.dma_start(out=wa, in_=wa_d[:, :])
            xtk = sb.tile([D, KV_TILES * 128], BF16)
            nc.sync.dma_start(out=xtk[:, 0:CHUNK], in_=xtk_d[:, 0:CHUNK])
            xtq = sb.tile([D, T], BF16)
            nc.sync.dma_start(out=xtq[:, 0:CHUNK], in_=xtq_d[:, 0:CHUNK])
            wb = sb.tile([D, 2 * D + 4 * 128], BF16)
            nc.sync.dma_start(out=wb, in_=wb_d[:, :])
            # bulk x, split and ordered by first use
            nc.sync.dma_start(out=xtk[:, CHUNK:2 * CHUNK],
                              in_=xtk_d[:, CHUNK:2 * CHUNK])
            nc.sync.dma_start(out=xtk[:, 2 * CHUNK:4 * CHUNK],
                              in_=xtk_d[:, 2 * CHUNK:4 * CHUNK])
            nc.sync.dma_start(out=xtq[:, CHUNK:4 * CHUNK],
                              in_=xtq_d[:, CHUNK:4 * CHUNK])
            nc.sync.dma_start(out=xtk[:, 4 * CHUNK:], in_=xtk_d[:, 4 * CHUNK:])
            nc.sync.dma_start(out=xtq[:, 4 * CHUNK:], in_=xtq_d[:, 4 * CHUNK:])
            wk = wa[:, 0:D]
            wq = wa[:, D:2 * D]
            bq = wa[:, 2 * D:2 * D + 2].bitcast(F32)
            wv = wb[:, 0:D]
            ident = wb[:, D:2 * D]
            msk = wb[:, 2 * D:]

            qt = sb.tile([D, T], BF16)                # Q^T (scaled, biased)
            kt = sb.tile([D, KV_TILES * 128], BF16)   # K^T
            vsb = sb.tile([D, KV_TILES * 128], BF16)  # V tiles [kvrow, e]

            pt_tiles = [ptp.tile([D, 2 * CHUNK], BF16, name=f"pt{i}")
                        for i in range(4)]
            npt = [0]
            ntmp = [0]

            # ---- emission helpers ----
            def eng_copy(eng, out, in_):
                if eng is nc.scalar:
                    nc.scalar.copy(out, in_)
                else:
                    (eng or nc.vector).tensor_copy(out, in_)

            def proj_K(g, eng=None, half=None):
                for h in ((0, 1) if half is None else (half,)):
                    sl = slice(g * CHUNK + h * 256, g * CHUNK + (h + 1) * 256)
                    ps = prj.tile([D, 256], F32, tag="prj", name="prjk")
                    nc.tensor.matmul(ps, wk, xtk[:, sl],
                                     start=True, stop=True,
                                     skip_group_check=True)
                    eng_copy(eng, kt[:, sl], ps)

            def proj_V(g, eng=None, half=None):
                for h in ((0, 1) if half is None else (half,)):
                    ps = prj.tile([D, 256], F32, tag="prj", name="prjv")
                    for jj in range(2):
                        t = 4 * g + 2 * h + jj
                        nc.tensor.matmul(
                            ps[:, jj * 128:(jj + 1) * 128],
                            xtk[:, t * 128:(t + 1) * 128], wv,
                            start=True, stop=True, skip_group_check=True)
                    sl = slice(g * CHUNK + h * 256, g * CHUNK + (h + 1) * 256)
                    eng_copy(eng, vsb[:, sl], ps)

            def proj_Q(g, eng=None, half=None):
                for h in ((0, 1) if half is None else (half,)):
                    sl = slice(g * CHUNK + h * 256, g * CHUNK + (h + 1) * 256)
                    ps = prj.tile([D, 256], F32, tag="prj", name="prjq")
                    nc.tensor.matmul(ps, wq, xtq[:, sl],
                                     start=True, stop=True,
                                     skip_group_check=True)
                    if eng is nc.scalar:
                        nc.scalar.activation(
                            qt[:, sl], ps,
                            mybir.ActivationFunctionType.Identity, bias=bq)
                    else:
                        nc.vector.tensor_scalar_add(qt[:, sl], ps, bq)

            state = {"pend": None, "acc": {}, "psum": {}, "epi": [],
                     "projplan": {}, "uidx": 0}

            def emit_epilogue(final=False):
                c, po, psum_t = state["epi"].pop(0)
                qsl = slice(c * CHUNK, (c + 1) * CHUNK)
                # lsum is ready at presum-chain end; ship it before po's copy
                nc.sync.dma_start(out=ls_d[:, qsl], in_=psum_t)
                ob = osb.tile([D, CHUNK], BF16, tag="ob", name="ob")
                if final:
                    nc.scalar.copy(ob, po)   # ACT is idle after the last exp
                else:
                    nc.vector.tensor_copy(ob, po)
                nc.sync.dma_start(out=ot_d[:, qsl], in_=ob)

            def emit_av(pend):
                c, ts, pair, is_last, pt, los, cum = pend
                if c not in state["acc"]:
                    state["acc"][c] = op.tile([D, CHUNK], F32, tag="po",
                                              name="po")
                po = state["acc"][c]
                for i, t in enumerate(pair):
                    lo = los[i]
                    ptc = pt[:, cum[i]:cum[i + 1]]
                    nc.tensor.matmul(
                        po[:, lo:], vsb[:, t * 128:(t + 1) * 128], ptc,
                        start=(t == ts[0]), stop=(t == ts[-1]),
                        skip_group_check=True)
                if is_last:
                    state["epi"].append((c, po, state["psum"][c]))
                    del state["acc"][c]
                    del state["psum"][c]

            def emit_unit(c, ts, pair, is_last, ui):
                """ST + mask (PE), exp (ACT), presum (DVE), delayed AV."""
                if state["epi"]:
                    emit_epilogue()
                los = [128 * (t - 4 * c) if c < 4 and t >= 4 * c else 0
                       for t in pair]
                # pack each tile's valid columns contiguously: tile i starts
                # at cum[i], width 512-los[i]; one exp covers [0:cum_end]
                cum = [0]
                for lo in los:
                    cum.append(cum[-1] + CHUNK - lo)
                st = stp.tile([D, 2 * CHUNK], F32, tag="st", name="st")
                for i, t in enumerate(pair):
                    lo = los[i]
                    nc.tensor.matmul(
                        st[:, cum[i]:cum[i + 1]],
                        kt[:, t * 128:(t + 1) * 128],
                        qt[:, c * CHUNK + lo:(c + 1) * CHUNK],
                        start=True, stop=True, skip_group_check=True)
                    if c < 4 and t >= 4 * c:
                        nc.tensor.matmul(
                            st[:, cum[i]:cum[i] + 128],
                            ident,
                            msk[:, (t - 4 * c) * 128:(t - 4 * c + 1) * 128],
                            start=False, stop=True, skip_group_check=True)
                pt = pt_tiles[npt[0] % 4]
                npt[0] += 1
                nc.scalar.activation(
                    pt[:, 0:cum[-1]], st[:, 0:cum[-1]],
                    mybir.ActivationFunctionType.Exp)

                # ---- denominator presum into ptsum (bf16) ----
                # tree: tmp = ptA + ptB (Pool/DVE alternating), then the
                # short serial chain ptsum += tmp stays on DVE.
                if ui == 0:
                    psum_t = pts.tile([D, CHUNK], BF16, tag="pts",
                                      name="pts")
                    state["psum"][c] = psum_t
                psum_t = state["psum"][c]
                add = mybir.AluOpType.add
                if los[1] > 0:
                    # diagonal unit: small windowed ops straight on DVE
                    if ui == 0:
                        nc.vector.tensor_copy(
                            psum_t[:, 0:los[1]], pt[:, 0:los[1]])
                        nc.vector.tensor_tensor(
                            out=psum_t[:, los[1]:],
                            in0=pt[:, los[1]:cum[1]],
                            in1=pt[:, cum[1]:cum[2]], op=add)
                    else:
                        for i in range(len(pair)):
                            lo = los[i]
                            nc.vector.tensor_tensor(
                                out=psum_t[:, lo:], in0=psum_t[:, lo:],
                                in1=pt[:, cum[i]:cum[i + 1]],
                                op=add)
                elif ui == 0:
                    nc.vector.tensor_tensor(
                        out=psum_t, in0=pt[:, 0:CHUNK],
                        in1=pt[:, CHUNK:], op=add)
                else:
                    tmp = tmpp.tile([D, CHUNK], BF16, tag="tmp", name="tmp")
                    eng = nc.gpsimd if ntmp[0] % 2 == 0 else nc.vector
                    ntmp[0] += 1
                    eng.tensor_tensor(out=tmp, in0=pt[:, 0:CHUNK],
                                      in1=pt[:, CHUNK:], op=add)
                    nc.vector.tensor_tensor(out=psum_t, in0=psum_t,
                                            in1=tmp, op=add)

                for fn in state["projplan"].get(state["uidx"], ()):
                    fn()
                state["uidx"] += 1
                prev, state["pend"] = state["pend"], (c, ts, pair, is_last,
                                                      pt, los, cum)
                if prev is not None:
                    emit_av(prev)

            def emit_chunk(c):
                ts = chunk_ts[c]
                pairs = [ts[i:i + 2] for i in range(0, len(ts), 2)]
                for pi, pair in enumerate(pairs):
                    emit_unit(c, ts, pair, pi == len(pairs) - 1, pi)

            # ---- PE warmup: back-to-back dummy matmuls from t~0.6us keep
            # the tensor engine's p-state ramp running during the input DMA
            # wait, so real projections start at full clock. Operands are
            # uninitialized SBUF (qt is first WRITTEN later); results land in
            # prj tiles that are overwritten with start=True. ----
            scr = sb.tile([D, 384], BF16)
            nc.gpsimd.memset(scr, 0.0)
            for _ in range(12):
                wup = prj.tile([D, CHUNK], F32, tag="prj", name="wup")
                nc.tensor.matmul(wup[:, 0:256], scr[:, 0:D],
                                 scr[:, D:D + 256],
                                 start=True, stop=True, skip_group_check=True)

            # ---- interleaved schedule ----
            # Group-0 projections run up front (qt0's copy rides the idle
            # ACT so kt0 on DVE lands in parallel). Every other projection
            # piece is queued and dropped one-per-unit into the attention
            # stream, ordered by first use. Chunk 3 runs last so the kernel
            # tail is a small masked diagonal unit, not a full one.
            proj_K(0, "split"); proj_Q(0, "split")
            # static plan: unit index -> proj half-pieces to emit there,
            # each ~2-4 units ahead of first use (chunk order 0,1,2,4,5,6,7,3)
            fns = {"Q": proj_Q, "K": proj_K, "V": proj_V}
            plan = {
                0: [("Q", 1, 0), ("Q", 1, 1)],
                1: [("K", 1, 0)],
                2: [("K", 1, 1), ("V", 1, 0)],
                3: [("V", 1, 1), ("Q", 2, 0)],
                4: [("Q", 2, 1)],
                6: [("K", 2, 0)],
                7: [("K", 2, 1), ("V", 2, 0)],
                8: [("V", 2, 1), ("Q", 4, 0)],
                9: [("Q", 4, 1), ("K", 5, 1)],
                10: [("K", 5, 0), ("V", 5, 1)],
                11: [("V", 5, 0), ("K", 4, 1)],
                12: [("K", 4, 0), ("V", 4, 1)],
                13: [("V", 4, 0), ("Q", 5, 0)],
                14: [("Q", 5, 1)],
                16: [("Q", 6, 0)],
                17: [("Q", 6, 1)],
                20: [("Q", 7, 0)],
                21: [("Q", 7, 1)],
                26: [("Q", 3, 0)],
                27: [("Q", 3, 1)],
                28: [("K", 3, 0)],
                29: [("K", 3, 1)],
                30: [("V", 3, 0)],
                31: [("V", 3, 1)],
            }
            state["projplan"] = {
                u: [(lambda f=fns[k], g=g, h=h: f(g, None, h))
                    for k, g, h in pieces]
                for u, pieces in plan.items()
            }
            for c in (0, 1, 2, 4, 5, 6, 7, 3):
                emit_chunk(c)
            emit_av(state["pend"])
            while state["epi"]:
                emit_epilogue(final=len(state["epi"]) == 1)

    if legalize:
        _legalize_multiwaits(nc)
    nc.finalize()
    return nc


def _legalize_multiwaits(nc):
    """Hardware instruction structs accept at most ONE sync wait. Move all
    but the last wait onto single-wait same-engine NoOps inserted before the
    instruction (same-engine program order preserves semantics)."""
    for fn in nc.m.functions:
        for blk in fn.blocks:
            insts = blk.instructions
            out = []
            for inst in insts:
                si = inst.sync_info
                if si is not None and si.on_wait and len(si.on_wait) >= 2:
                    waits = list(si.on_wait)
                    for w in waits[:-1]:
                        out.append(mybir.InstNoOp(
                            name=nc.get_next_instruction_name(),
                            engine=inst.engine,
                            bass_nofuse=True,
                            sync_info=mybir.SyncInfo(
                                on_wait=[w], on_update=[]),
                        ))
                    inst.sync_info = mybir.SyncInfo(
                        on_wait=[waits[-1]],
                        on_update=list(si.on_update or []))
                out.append(inst)
            insts[:] = out


_NC_CACHE = {}


def get_nc(legalize=True):
    key = ("nc", legalize)
    if key not in _NC_CACHE:
        _NC_CACHE[key] = build_nc(legalize)
    return _NC_CACHE[key]


def make_core_inputs(x, Wq, bq, Wk, bk, Wv, bv):
    """Per-core input maps (host-side sharding). bk drops out of softmax;
    bv is applied on the host."""
    s = 1.0 / math.sqrt(D)
    wq_s = (np.asarray(Wq, np.float32) * s).astype(bfloat16)
    bq_s = (np.asarray(bq, np.float32) * s)
    wk = np.asarray(Wk, np.float32).astype(bfloat16)
    wv = np.asarray(Wv, np.float32).astype(bfloat16)

    qp = np.arange(128)[None, :]
    kk = np.arange(128)[:, None]
    # per-m staircase band (query cols [128m, 128m+128) relative part)
    msk = np.concatenate(
        [np.where(qp >= kk, 0.0, NEG) for m in range(4)],
        axis=1).astype(bfloat16)
    ident = np.eye(D, dtype=np.float32).astype(bfloat16)
    bq_bits = np.ascontiguousarray(bq_s[:, None]).view(bfloat16)
    wpackA = np.concatenate([wk, wq_s, bq_bits], axis=1)
    wpackB = np.concatenate([wv, ident, msk], axis=1)

    x = np.asarray(x, dtype=np.float32)
    in_maps = []
    for core in range(8):
        b, h = core // 2, core % 2
        xb = x[b]                                   # [4096, 128]
        tri = xb[h * HALF:(h + 1) * HALF]           # [2048, 128]
        rect_q = xb[HALF:]                          # [2048, 128]
        rect_kv = xb[h * 1024:(h + 1) * 1024]       # [1024, 128]
        xtq = np.ascontiguousarray(
            np.concatenate([tri, rect_q], axis=0).T).astype(bfloat16)
        xtk = np.ascontiguousarray(
            np.concatenate([tri, rect_kv], axis=0).T).astype(bfloat16)
        in_maps.append({
            "xTq": xtq, "xTk": xtk, "wpackA": wpackA, "wpackB": wpackB,
        })
    return in_maps


def merge_outputs(results, bv):
    """Gather per-core (oT, lsum) into the full [B, T, D] output."""
    bv = np.asarray(bv, dtype=np.float32)
    out = np.empty((B, T, D), np.float32)
    for b in range(B):
        lo, hi = results[2 * b], results[2 * b + 1]
        lo_oT = np.asarray(lo["oT"], np.float64)
        hi_oT = np.asarray(hi["oT"], np.float64)
        lo_l = np.asarray(lo["lsum"], np.float64).sum(axis=0)   # [T]
        hi_l = np.asarray(hi["lsum"], np.float64).sum(axis=0)   # [T]
        O = np.zeros((T, D), np.float64)
        L = np.zeros(T, np.float64)
        O[:HALF] += lo_oT[:, :HALF].T
        L[:HALF] += lo_l[:HALF]
        O[HALF:] += hi_oT[:, :HALF].T
        L[HALF:] += hi_l[:HALF]
        O[HALF:] += lo_oT[:, HALF:].T
        L[HALF:] += lo_l[HALF:]
        O[HALF:] += hi_oT[:, HALF:].T
        L[HALF:] += hi_l[HALF:]
        out[b] = (O / L[:, None]).astype(np.float32) + bv
    return out


def run_per_core(nc, in_maps, threads=True):
    """Run the same single-core program on each NeuronCore with its own
    inputs (independent dispatch; the cores share no collectives)."""
    import jax
    from concourse import bass2jax

    devices = jax.devices()[:len(in_maps)]

    def one(i):
        with jax.default_device(devices[i]):
            return bass2jax.run_bass_via_pjrt(nc, [in_maps[i]], n_cores=1)[0]

    if threads:
        from concurrent.futures import ThreadPoolExecutor
        first = one(0)
        with ThreadPoolExecutor(max_workers=7) as ex:
            rest = list(ex.map(one, range(1, len(in_maps))))
        return [first] + rest
    return [one(i) for i in range(len(in_maps))]


def kernel(x, Wq, bq, Wk, bk, Wv, bv, _trace=False):
    from concourse.bass_utils import axon_active, run_bass_kernel_spmd

    nc = get_nc()
    in_maps = make_core_inputs(x, Wq, bq, Wk, bk, Wv, bv)
    if axon_active():
        results = run_per_core(nc, in_maps)
    else:
        res = run_bass_kernel_spmd(nc, in_maps, list(range(8)), trace=_trace)
        kernel.last_result = res
        results = res.results
    out = merge_outputs(results, bv)
    return out


# revision 29
# speedup vs baseline: 1.0039x; 1.0009x over previous
"""Trainium2 Bass kernel v3: single-head causal attention, bf16 datapath.

Problem: x[4,4096,128]; Q/K/V linear projections (W [in,out] layout, +bias);
scores = QK^T/sqrt(128) with causal mask; softmax; out = P @ V.

Sharding (8 cores = 4 batches x 2), identical to v2:
  core (b, h):
    triangle: queries [2048h, 2048h+2048) of batch b attending causally
        within the same range.
    rectangle: queries [2048, 4096) attending to kv rows [1024h, 1024h+1024).
  Union over a batch's two cores covers the causal set exactly once.

Softmax without max subtraction (scores ~N(0,1)); cross-core merge is
linear: host sums unnormalized outputs oT and denominators, then divides.

v3 changes vs v2 (cost-model-driven):
  - bf16 everywhere on the datapath (x, W, Q^T, K^T, V, P~, mask) instead of
    fp32r: halves DMA traffic and SBUF; matmul rate identical (1 cyc/row).
  - NO on-device softmax denominator: the PE "ones" matmuls (l) are gone.
    Instead the DVE accumulates each chunk's exp'd probability tiles into
    ptsum[128, 512] (bf16) and the host reduces the 128 kv-lanes. This takes
    ~14us of matmul time off the PE (the former bottleneck) for ~20us of
    otherwise-idle DVE time.
  - projections interleaved with attention chunks: the ACT engine (exp) and
    the DMA start ~2.5us into the kernel instead of after the whole
    projection phase.
  - epilogue: po (PSUM) -> bf16 SBUF copy -> DMA; ptsum DMAs straight from
    SBUF. Outputs oT[D,T] bf16 + lsum[D,T] bf16.

Per-unit steady state (pair of kv tiles x 512 queries):
  PE: ST pair (1024 rows) + AV pair (1024 rows) ~ 854ns @full clock
  ACT: exp [128, <=1024] ~ 1038ns  <- pacer
  DVE: 2 presum adds ~ 654ns (+ proj copies early on)
Engine budgets/core: PE ~37us (incl ~4us p-state ramp), ACT ~35us, DVE ~38us.
"""

import math
import sys

import numpy as np

sys.path.insert(0, "/opt/trn_rl_repo")

import ml_dtypes  # noqa: E402

import concourse.bass as bass  # noqa: E402
import concourse.mybir as mybir  # noqa: E402
from concourse.tile import TileContext  # noqa: E402

B, T, D = 4, 4096, 128
HALF = T // 2          # 2048 queries per triangle
NCHUNK = 8             # 8 chunks of 512 query slots per core (4 tri + 4 rect)
CHUNK = 512
KV_TRI_TILES = 16      # triangle kv tiles (2048 rows)
KV_RECT_TILES = 8      # rectangle kv tiles (1024 rows)
KV_TILES = KV_TRI_TILES + KV_RECT_TILES          # 24 tiles = 3072 kv rows
NEG = -1.0e5           # additive mask value; exp(NEG) == 0.0

F32 = mybir.dt.float32
BF16 = mybir.dt.bfloat16
bfloat16 = ml_dtypes.bfloat16


def build_nc(legalize=True):
    nc = bass.Bass()

    xtq_d = nc.declare_dram_parameter("xTq", [D, T], BF16, isOutput=False)
    xtk_d = nc.declare_dram_parameter("xTk", [D, KV_TILES * 128], BF16, isOutput=False)
    wa_d = nc.declare_dram_parameter("wpackA", [D, 258], BF16, isOutput=False)
    wb_d = nc.declare_dram_parameter("wpackB", [D, 2 * D + 4 * 128], BF16,
                                     isOutput=False)

    ot_d = nc.declare_dram_parameter("oT", [D, T], BF16, isOutput=True)
    ls_d = nc.declare_dram_parameter("lsum", [D, T], BF16, isOutput=True)

    # attention chunk -> kv tile order: diagonal tiles ascending (valid-col
    # prefix logic needs the widest first), then full tiles in reverse so the
    # masked diagonal work lands right after the chunk's projections.
    def _tri_ts(c):
        fulls = list(range(0, 4 * c))[::-1]
        diag = list(range(4 * c, 4 * c + 4))
        # keep one full pair after the diagonals so the chunk's last exp is
        # full-sized and covers the boundary refill; chunk 3 runs last in the
        # kernel and keeps diag-last so the kernel tail stays small
        if len(fulls) >= 2 and c != 3:
            return fulls[:-2] + diag + fulls[-2:]
        return fulls + diag

    chunk_ts = [_tri_ts(c) for c in range(4)] + \
               [list(range(16, 24))[::-1] for _ in range(4)]

    with TileContext(nc) as tc:
        with (
            tc.tile_pool(name="sb", bufs=1) as sb,          # resident tensors
            tc.tile_pool(name="stp", bufs=2, space="PSUM") as stp,   # 4 banks
            tc.tile_pool(name="prj", bufs=2, space="PSUM") as prj,   # 2 banks
            tc.tile_pool(name="op", bufs=2, space="PSUM") as op,     # 2 banks
            tc.tile_pool(name="ptp", bufs=1) as ptp,
            tc.tile_pool(name="pts", bufs=3) as pts,
            tc.tile_pool(name="tmpp", bufs=6) as tmpp,
            tc.tile_pool(name="osb", bufs=3) as osb,
        ):
            # ---- resident SBUF tensors; DMAs issued in first-use order ----
            wa = sb.tile([D, 258], BF16)
            nc.sync.dma_start(out=wa, in_=wa_d[:, :])
            xtk = sb.tile([D, KV_TILES * 128], BF16)
            nc.sync.dma_start(out=xtk[:, 0:CHUNK], in_=xtk_d[:, 0:CHUNK])
            xtq = sb.tile([D, T], BF16)
            nc.sync.dma_start(out=xtq[:, 0:CHUNK], in_=xtq_d[:, 0:CHUNK])
            wb = sb.tile([D, 2 * D + 4 * 128], BF16)
            nc.sync.dma_start(out=wb, in_=wb_d[:, :])
            # bulk x, split and ordered by first use
            nc.sync.dma_start(out=xtk[:, CHUNK:2 * CHUNK],
                              in_=xtk_d[:, CHUNK:2 * CHUNK])
            nc.sync.dma_start(out=xtk[:, 2 * CHUNK:4 * CHUNK],
                              in_=xtk_d[:, 2 * CHUNK:4 * CHUNK])
            nc.sync.dma_start(out=xtq[:, CHUNK:4 * CHUNK],
                              in_=xtq_d[:, CHUNK:4 * CHUNK])
            nc.sync.dma_start(out=xtk[:, 4 * CHUNK:], in_=xtk_d[:, 4 * CHUNK:])
            nc.sync.dma_start(out=xtq[:, 4 * CHUNK:], in_=xtq_d[:, 4 * CHUNK:])
            wk = wa[:, 0:D]
            wq = wa[:, D:2 * D]
            bq = wa[:, 2 * D:2 * D + 2].bitcast(F32)
            wv = wb[:, 0:D]
            ident = wb[:, D:2 * D]
            msk = wb[:, 2 * D:]

            qt = sb.tile([D, T], BF16)                # Q^T (scaled, biased)
            kt = sb.tile([D, KV_TILES * 128], BF16)   # K^T
            vsb = sb.tile([D, KV_TILES * 128], BF16)  # V tiles [kvrow, e]

            pt_tiles = [ptp.tile([D, 2 * CHUNK], BF16, name=f"pt{i}")
                        for i in range(4)]
            npt = [0]
            ntmp = [0]

            # ---- emission helpers ----
            def eng_copy(eng, out, in_):
                if eng is nc.scalar:
                    nc.scalar.copy(out, in_)
                else:
                    (eng or nc.vector).tensor_copy(out, in_)

            def proj_K(g, eng=None, half=None):
                for h in ((0, 1) if half is None else (half,)):
                    sl = slice(g * CHUNK + h * 256, g * CHUNK + (h + 1) * 256)
                    ps = prj.tile([D, 256], F32, tag="prj", name="prjk")
                    nc.tensor.matmul(ps, wk, xtk[:, sl],
                                     start=True, stop=True,
                                     skip_group_check=True)
                    eng_copy(eng, kt[:, sl], ps)

            def proj_V(g, eng=None, half=None):
                for h in ((0, 1) if half is None else (half,)):
                    ps = prj.tile([D, 256], F32, tag="prj", name="prjv")
                    for jj in range(2):
                        t = 4 * g + 2 * h + jj
                        nc.tensor.matmul(
                            ps[:, jj * 128:(jj + 1) * 128],
                            xtk[:, t * 128:(t + 1) * 128], wv,
                            start=True, stop=True, skip_group_check=True)
                    sl = slice(g * CHUNK + h * 256, g * CHUNK + (h + 1) * 256)
                    eng_copy(eng, vsb[:, sl], ps)

            def proj_Q(g, eng=None, half=None):
                for h in ((0, 1) if half is None else (half,)):
                    sl = slice(g * CHUNK + h * 256, g * CHUNK + (h + 1) * 256)
                    ps = prj.tile([D, 256], F32, tag="prj", name="prjq")
                    nc.tensor.matmul(ps, wq, xtq[:, sl],
                                     start=True, stop=True,
                                     skip_group_check=True)
                    if eng is nc.scalar:
                        nc.scalar.activation(
                            qt[:, sl], ps,
                            mybir.ActivationFunctionType.Identity, bias=bq)
                    else:
                        nc.vector.tensor_scalar_add(qt[:, sl], ps, bq)

            state = {"pend": None, "acc": {}, "psum": {}, "epi": [],
                     "projplan": {}, "uidx": 0}

            def emit_epilogue(final=False):
                c, po, psum_t = state["epi"].pop(0)
                qsl = slice(c * CHUNK, (c + 1) * CHUNK)
                # lsum is ready at presum-chain end; ship it before po's copy
                nc.sync.dma_start(out=ls_d[:, qsl], in_=psum_t)
                ob = osb.tile([D, CHUNK], BF16, tag="ob", name="ob")
                if final:
                    nc.scalar.copy(ob, po)   # ACT is idle after the last exp
                else:
                    nc.vector.tensor_copy(ob, po)
                nc.sync.dma_start(out=ot_d[:, qsl], in_=ob)

            def emit_av(pend):
                c, ts, pair, is_last, pt, los, cum = pend
                if c not in state["acc"]:
                    state["acc"][c] = op.tile([D, CHUNK], F32, tag="po",
                                              name="po")
                po = state["acc"][c]
                for i, t in enumerate(pair):
                    lo = los[i]
                    ptc = pt[:, cum[i]:cum[i + 1]]
                    nc.tensor.matmul(
                        po[:, lo:], vsb[:, t * 128:(t + 1) * 128], ptc,
                        start=(t == ts[0]), stop=(t == ts[-1]),
                        skip_group_check=True)
                if is_last:
                    state["epi"].append((c, po, state["psum"][c]))
                    del state["acc"][c]
                    del state["psum"][c]

            def emit_unit(c, ts, pair, is_last, ui):
                """ST + mask (PE), exp (ACT), presum (DVE), delayed AV."""
                if state["epi"]:
                    emit_epilogue()
                los = [128 * (t - 4 * c) if c < 4 and t >= 4 * c else 0
                       for t in pair]
                # pack each tile's valid columns contiguously: tile i starts
                # at cum[i], width 512-los[i]; one exp covers [0:cum_end]
                cum = [0]
                for lo in los:
                    cum.append(cum[-1] + CHUNK - lo)
                st = stp.tile([D, 2 * CHUNK], F32, tag="st", name="st")
                for i, t in enumerate(pair):
                    lo = los[i]
                    nc.tensor.matmul(
                        st[:, cum[i]:cum[i + 1]],
                        kt[:, t * 128:(t + 1) * 128],
                        qt[:, c * CHUNK + lo:(c + 1) * CHUNK],
                        start=True, stop=True, skip_group_check=True)
                    if c < 4 and t >= 4 * c:
                        nc.tensor.matmul(
                            st[:, cum[i]:cum[i] + 128],
                            ident,
                            msk[:, (t - 4 * c) * 128:(t - 4 * c + 1) * 128],
                            start=False, stop=True, skip_group_check=True)
                pt = pt_tiles[npt[0] % 4]
                npt[0] += 1
                nc.scalar.activation(
                    pt[:, 0:cum[-1]], st[:, 0:cum[-1]],
                    mybir.ActivationFunctionType.Exp)

                # ---- denominator presum into ptsum (bf16) ----
                # tree: tmp = ptA + ptB (Pool/DVE alternating), then the
                # short serial chain ptsum += tmp stays on DVE.
                if ui == 0:
                    psum_t = pts.tile([D, CHUNK], BF16, tag="pts",
                                      name="pts")
                    state["psum"][c] = psum_t
                psum_t = state["psum"][c]
                add = mybir.AluOpType.add
                if los[1] > 0:
                    # diagonal unit: small windowed ops straight on DVE
                    if ui == 0:
                        nc.vector.tensor_copy(
                            psum_t[:, 0:los[1]], pt[:, 0:los[1]])
                        nc.vector.tensor_tensor(
                            out=psum_t[:, los[1]:],
                            in0=pt[:, los[1]:cum[1]],
                            in1=pt[:, cum[1]:cum[2]], op=add)
                    else:
                        for i in range(len(pair)):
                            lo = los[i]
                            nc.vector.tensor_tensor(
                                out=psum_t[:, lo:], in0=psum_t[:, lo:],
                                in1=pt[:, cum[i]:cum[i + 1]],
                                op=add)
                elif ui == 0:
                    nc.vector.tensor_tensor(
                        out=psum_t, in0=pt[:, 0:CHUNK],
                        in1=pt[:, CHUNK:], op=add)
                else:
                    tmp = tmpp.tile([D, CHUNK], BF16, tag="tmp", name="tmp")
                    # the chunk epilogue waits on the final chain add, so the
                    # last unit's pair-sum always rides the fast DVE path
                    if is_last:
                        eng = nc.vector
                    else:
                        eng = nc.gpsimd if ntmp[0] % 2 == 0 else nc.vector
                    ntmp[0] += 1
                    eng.tensor_tensor(out=tmp, in0=pt[:, 0:CHUNK],
                                      in1=pt[:, CHUNK:], op=add)
                    nc.vector.tensor_tensor(out=psum_t, in0=psum_t,
                                            in1=tmp, op=add)

                for fn in state["projplan"].get(state["uidx"], ()):
                    fn()
                state["uidx"] += 1
                prev, state["pend"] = state["pend"], (c, ts, pair, is_last,
                                                      pt, los, cum)
                if prev is not None:
                    emit_av(prev)

            def emit_chunk(c):
                ts = chunk_ts[c]
                pairs = [ts[i:i + 2] for i in range(0, len(ts), 2)]
                for pi, pair in enumerate(pairs):
                    emit_unit(c, ts, pair, pi == len(pairs) - 1, pi)

            # ---- PE warmup: back-to-back dummy matmuls from t~0.6us keep
            # the tensor engine's p-state ramp running during the input DMA
            # wait, so real projections start at full clock. Operands are
            # uninitialized SBUF (qt is first WRITTEN later); results land in
            # prj tiles that are overwritten with start=True. ----
            scr = sb.tile([D, 384], BF16)
            nc.gpsimd.memset(scr, 0.0)
            for _ in range(12):
                wup = prj.tile([D, CHUNK], F32, tag="prj", name="wup")
                nc.tensor.matmul(wup[:, 0:256], scr[:, 0:D],
                                 scr[:, D:D + 256],
                                 start=True, stop=True, skip_group_check=True)

            # ---- interleaved schedule ----
            # Group-0 projections run up front (qt0's copy rides the idle
            # ACT so kt0 on DVE lands in parallel). Every other projection
            # piece is queued and dropped one-per-unit into the attention
            # stream, ordered by first use. Chunk 3 runs last so the kernel
            # tail is a small masked diagonal unit, not a full one.
            proj_K(0, "split"); proj_Q(0, "split")
            # static plan: unit index -> proj half-pieces to emit there,
            # each ~2-4 units ahead of first use (chunk order 0,1,2,4,5,6,7,3)
            fns = {"Q": proj_Q, "K": proj_K, "V": proj_V}
            plan = {
                0: [("Q", 1, 0), ("Q", 1, 1)],
                1: [("K", 1, 0)],
                2: [("K", 1, 1), ("V", 1, 0)],
                3: [("V", 1, 1), ("Q", 2, 0)],
                4: [("Q", 2, 1)],
                6: [("K", 2, 0)],
                7: [("K", 2, 1), ("V", 2, 0)],
                8: [("V", 2, 1), ("Q", 4, 0)],
                9: [("Q", 4, 1), ("K", 5, 1)],
                10: [("K", 5, 0), ("V", 5, 1)],
                11: [("V", 5, 0), ("K", 4, 1)],
                12: [("K", 4, 0), ("V", 4, 1)],
                13: [("V", 4, 0), ("Q", 5, 0)],
                14: [("Q", 5, 1)],
                16: [("Q", 6, 0)],
                17: [("Q", 6, 1)],
                20: [("Q", 7, 0)],
                21: [("Q", 7, 1)],
                26: [("Q", 3, 0)],
                27: [("Q", 3, 1)],
                28: [("K", 3, 0)],
                29: [("K", 3, 1)],
                30: [("V", 3, 0)],
                31: [("V", 3, 1)],
            }
            state["projplan"] = {
                u: [(lambda f=fns[k], g=g, h=h: f(g, None, h))
                    for k, g, h in pieces]
                for u, pieces in plan.items()
            }
            for c in (0, 1, 2, 4, 5, 6, 7, 3):
                emit_chunk(c)
            emit_av(state["pend"])
            while state["epi"]:
                emit_epilogue(final=len(state["epi"]) == 1)

    if legalize:
        _legalize_multiwaits(nc)
    nc.finalize()
    return nc


def _legalize_multiwaits(nc):
    """Hardware instruction structs accept at most ONE sync wait. Move all
    but the last wait onto single-wait same-engine NoOps inserted before the
    instruction (same-engine program order preserves semantics)."""
    for fn in nc.m.functions:
        for blk in fn.blocks:
            insts = blk.instructions
            out = []
            for inst in insts:
                si = inst.sync_info
                if si is not None and si.on_wait and len(si.on_wait) >= 2:
                    waits = list(si.on_wait)
                    for w in waits[:-1]:
                        out.append(mybir.InstNoOp(
                            name=nc.get_next_instruction_name(),
                            engine=inst.engine,
                            bass_nofuse=True,
                            sync_info=mybir.SyncInfo(
                                on_wait=[w], on_update=[]),
                        ))
                    inst.sync_info = mybir.SyncInfo(
                        on_wait=[waits[-1]],
                        on_update=list(si.on_update or []))
                out.append(inst)
            insts[:] = out


_NC_CACHE = {}


def get_nc(legalize=True):
    key = ("nc", legalize)
    if key not in _NC_CACHE:
        _NC_CACHE[key] = build_nc(legalize)
    return _NC_CACHE[key]


def make_core_inputs(x, Wq, bq, Wk, bk, Wv, bv):
    """Per-core input maps (host-side sharding). bk drops out of softmax;
    bv is applied on the host."""
    s = 1.0 / math.sqrt(D)
    wq_s = (np.asarray(Wq, np.float32) * s).astype(bfloat16)
    bq_s = (np.asarray(bq, np.float32) * s)
    wk = np.asarray(Wk, np.float32).astype(bfloat16)
    wv = np.asarray(Wv, np.float32).astype(bfloat16)

    qp = np.arange(128)[None, :]
    kk = np.arange(128)[:, None]
    # per-m staircase band (query cols [128m, 128m+128) relative part)
    msk = np.concatenate(
        [np.where(qp >= kk, 0.0, NEG) for m in range(4)],
        axis=1).astype(bfloat16)
    ident = np.eye(D, dtype=np.float32).astype(bfloat16)
    bq_bits = np.ascontiguousarray(bq_s[:, None]).view(bfloat16)
    wpackA = np.concatenate([wk, wq_s, bq_bits], axis=1)
    wpackB = np.concatenate([wv, ident, msk], axis=1)

    x = np.asarray(x, dtype=np.float32)
    in_maps = []
    for core in range(8):
        b, h = core // 2, core % 2
        xb = x[b]                                   # [4096, 128]
        tri = xb[h * HALF:(h + 1) * HALF]           # [2048, 128]
        rect_q = xb[HALF:]                          # [2048, 128]
        rect_kv = xb[h * 1024:(h + 1) * 1024]       # [1024, 128]
        xtq = np.ascontiguousarray(
            np.concatenate([tri, rect_q], axis=0).T).astype(bfloat16)
        xtk = np.ascontiguousarray(
            np.concatenate([tri, rect_kv], axis=0).T).astype(bfloat16)
        in_maps.append({
            "xTq": xtq, "xTk": xtk, "wpackA": wpackA, "wpackB": wpackB,
        })
    return in_maps


def merge_outputs(results, bv):
    """Gather per-core (oT, lsum) into the full [B, T, D] output."""
    bv = np.asarray(bv, dtype=np.float32)
    out = np.empty((B, T, D), np.float32)
    for b in range(B):
        lo, hi = results[2 * b], results[2 * b + 1]
        lo_oT = np.asarray(lo["oT"], np.float64)
        hi_oT = np.asarray(hi["oT"], np.float64)
        lo_l = np.asarray(lo["lsum"], np.float64).sum(axis=0)   # [T]
        hi_l = np.asarray(hi["lsum"], np.float64).sum(axis=0)   # [T]
        O = np.zeros((T, D), np.float64)
        L = np.zeros(T, np.float64)
        O[:HALF] += lo_oT[:, :HALF].T
        L[:HALF] += lo_l[:HALF]
        O[HALF:] += hi_oT[:, :HALF].T
        L[HALF:] += hi_l[:HALF]
        O[HALF:] += lo_oT[:, HALF:].T
        L[HALF:] += lo_l[HALF:]
        O[HALF:] += hi_oT[:, HALF:].T
        L[HALF:] += hi_l[HALF:]
        out[b] = (O / L[:, None]).astype(np.float32) + bv
    return out


def run_per_core(nc, in_maps, threads=True):
    """Run the same single-core program on each NeuronCore with its own
    inputs (independent dispatch; the cores share no collectives)."""
    import jax
    from concourse import bass2jax

    devices = jax.devices()[:len(in_maps)]

    def one(i):
        with jax.default_device(devices[i]):
            return bass2jax.run_bass_via_pjrt(nc, [in_maps[i]], n_cores=1)[0]

    if threads:
        from concurrent.futures import ThreadPoolExecutor
        first = one(0)
        with ThreadPoolExecutor(max_workers=7) as ex:
            rest = list(ex.map(one, range(1, len(in_maps))))
        return [first] + rest
    return [one(i) for i in range(len(in_maps))]


def kernel(x, Wq, bq, Wk, bk, Wv, bv, _trace=False):
    from concourse.bass_utils import axon_active, run_bass_kernel_spmd

    nc = get_nc()
    in_maps = make_core_inputs(x, Wq, bq, Wk, bk, Wv, bv)
    if axon_active():
        results = run_per_core(nc, in_maps)
    else:
        res = run_bass_kernel_spmd(nc, in_maps, list(range(8)), trace=_trace)
        kernel.last_result = res
        results = res.results
    out = merge_outputs(results, bv)
    return out
